# revision 19
# baseline (speedup 1.0000x reference)
"""EnhancedPolarAttention Trainium2 Bass kernel.

Full inputs in, full output out. Head-parallel across 8 NeuronCores
(1 head per core). See bottom of file for the host-side kernel() entry.

Math: scores = (q.k)/sqrt(hd) * r_w[j] * cos(theta_i - theta_j)
   with cos(a-b) = cos a cos b + sin a sin b, this is exactly
   q'_i . k'_j  with
   q' = [q * cos(theta_i), q * sin(theta_i)] / sqrt(hd)   (64-dim)
   k' = [k * r_w cos(theta_j), k * r_w sin(theta_j)]      (64-dim)
so the polar modulation folds into the QK^T matmul (contraction 64).

Scores are tiny (|s| < ~0.5), so softmax needs no max subtraction:
exp on ScalarE directly, and the denominator Z = sum_j exp(s_ij) is
obtained for free by augmenting v with a ones column inside the
attn @ v matmul (out row 32 = Z).

Everything is computed in a transposed (feature-major) layout so no
on-device transposes are needed at all:
  xT [C=128, N=4096]          (host pre-transposed, replicated)
  qk' [128, 4096]             rows 0-63 = q'T, rows 64-127 = k'T
  scoresT tile [128 keys, 512 queries] = k'chunk.T @ q'T  (PSUM)
  acc [33, 512] += v_aug_chunk.T @ exp(scoresT chunk)     (PSUM)
  outT [32, 512] = acc[0:32] * broadcast(1/acc[32])
  final [128 tok, 256] = outT_slice.T @ Wf_h  (+ bf/8)
Host sums the 8 per-head partial projections.
"""

import numpy as np

# ---- problem constants (hardcoded per contract) ----
B, HI, WI, C = 1, 64, 64, 128
N = HI * WI            # 4096
KEY_DIM = 256
NH = 8                 # heads
HD = KEY_DIM // NH     # 32
NCORES = 8
QC = 512               # query chunk = one PSUM bank of f32
NQC = N // QC          # 8
KC = 128               # key chunk = partition dim
NKC = N // KC          # 32
VW = HD + 1            # v augmented with ones column -> 33

_CACHE = {}


def _polar_constants():
    """Match reference._polar_constants in float32 numpy."""
    H, W = HI, WI
    y, x = np.meshgrid(np.arange(H, dtype=np.float32),
                       np.arange(W, dtype=np.float32))
    x = x.reshape(-1)
    y = y.reshape(-1)
    r = np.sqrt(np.square(x - W / 2) + np.square(y - H / 2)).astype(np.float32) + np.float32(1e-6)
    theta = np.arctan2(y - H / 2, x - W / 2).astype(np.float32)
    log_r = (np.log(r) / np.log(r.max())).astype(np.float32)
    theta = ((theta + 2 * np.pi) % (2 * np.pi)).astype(np.float32)
    r_weight = (1.0 / (log_r + 1.0)).astype(np.float32)
    return r_weight, theta


def _build_nc(debug_taps=False):
    import concourse.mybir as mybir
    import concourse.tile as tile
    from concourse import bacc

    F32 = mybir.dt.float32
    EXP = mybir.ActivationFunctionType.Exp

    nc = bacc.Bacc("TRN2", target_bir_lowering=False)
    dbg = {}
    if debug_taps:
        dbg["qp"] = nc.dram_tensor("dbg_qp", [64, N], F32, kind="ExternalOutput")
        dbg["kp"] = nc.dram_tensor("dbg_kp", [64, N], F32, kind="ExternalOutput")
        dbg["v"] = nc.dram_tensor("dbg_v", [128, NKC * VW], F32, kind="ExternalOutput")
        dbg["ex0"] = nc.dram_tensor("dbg_ex0", [128, 3 * QC], F32, kind="ExternalOutput")
        dbg["acc0"] = nc.dram_tensor("dbg_acc0", [VW, QC], F32, kind="ExternalOutput")
        dbg["recip0"] = nc.dram_tensor("dbg_recip0", [1, QC], F32, kind="ExternalOutput")
        dbg["rb0"] = nc.dram_tensor("dbg_rb0", [HD, QC], F32, kind="ExternalOutput")
        dbg["outT0"] = nc.dram_tensor("dbg_outT0", [HD, QC], F32, kind="ExternalOutput")

    xT_d = nc.dram_tensor("xT", [C, N], F32, kind="ExternalInput")
    mcq_d = nc.dram_tensor("mcq", [64, N], F32, kind="ExternalInput")
    mck_d = nc.dram_tensor("mck", [64, N], F32, kind="ExternalInput")
    wqq_d = nc.dram_tensor("wqq", [C, 64], F32, kind="ExternalInput")
    wkk_d = nc.dram_tensor("wkk", [C, 64], F32, kind="ExternalInput")
    wv_d = nc.dram_tensor("wv", [C, HD], F32, kind="ExternalInput")
    wf_d = nc.dram_tensor("wf", [HD, KEY_DIM], F32, kind="ExternalInput")
    out_d = nc.dram_tensor("out", [N, KEY_DIM], F32, kind="ExternalOutput")

    with tile.TileContext(nc) as tc, \
         tc.tile_pool(name="singles", bufs=1) as singles, \
         tc.tile_pool(name="work", bufs=2) as work, \
         tc.tile_pool(name="psum", bufs=2, space="PSUM") as psum:

        # ---- persistent SBUF ----
        xT_sb = singles.tile([C, N], F32)
        mcq_sb = singles.tile([64, N], F32)
        mck_sb = singles.tile([64, N], F32)
        qp_sb = singles.tile([64, N], F32)        # q'T
        kp_sb = singles.tile([64, N], F32)        # k'T
        v_sb = singles.tile([128, NKC * VW], F32)  # 32 chunks of [128, 33]
        wqq_sb = singles.tile([C, 64], F32)
        wkk_sb = singles.tile([C, 64], F32)
        wv_sb = singles.tile([C, HD], F32)
        wf_sb = singles.tile([HD, KEY_DIM], F32)

        nc.vector.memset(v_sb, 1.0)   # every 33rd column stays 1.0

        # ---- input DMAs (chunked so compute can start early) ----
        for i in range(NQC):
            s = slice(i * QC, (i + 1) * QC)
            nc.sync.dma_start(out=xT_sb[:, s], in_=xT_d[:, s])
            nc.sync.dma_start(out=mcq_sb[:, s], in_=mcq_d[:, s])
            nc.sync.dma_start(out=mck_sb[:, s], in_=mck_d[:, s])
        nc.sync.dma_start(out=wqq_sb, in_=wqq_d[:, :])
        nc.sync.dma_start(out=wkk_sb, in_=wkk_d[:, :])
        nc.sync.dma_start(out=wv_sb, in_=wv_d[:, :])
        nc.sync.dma_start(out=wf_sb, in_=wf_d[:, :])

        # ---- phase A: q'/k' projection with polar modulation fused ----
        # ps_q = Wqq.T @ xT -> [64 feat(q,q), 512 tok]; q' = (ps_q+bq)*mcq
        # mcq rows: cos/sqrt(hd) x32, sin/sqrt(hd) x32; mck: r*cos, r*sin
        for i in range(NQC):
            s = slice(i * QC, (i + 1) * QC)
            ps_q = psum.tile([64, QC], F32, tag="s")
            nc.tensor.matmul(ps_q, wqq_sb, xT_sb[:, s], start=True, stop=True)
            nc.vector.tensor_mul(qp_sb[:, s], ps_q, mcq_sb[:, s])
            ps_k = psum.tile([64, QC], F32, tag="s")
            nc.tensor.matmul(ps_k, wkk_sb, xT_sb[:, s], start=True, stop=True)
            nc.vector.tensor_mul(kp_sb[:, s], ps_k, mck_sb[:, s])

        # ---- phase A2: v projection (token-major directly) ----
        # v_chunk [128 tok, 32] = xT_chunk.T @ Wv ; +bias, into v_sb 33-blocks
        for j in range(NKC):
            ps_v = psum.tile([128, HD], F32, tag="s")
            nc.tensor.matmul(ps_v, xT_sb[:, j * KC:(j + 1) * KC], wv_sb,
                             start=True, stop=True)
            nc.vector.tensor_copy(v_sb[:, j * VW:j * VW + HD], ps_v)

        # ---- phase B: attention main loop ----
        # groups of 3 key-chunks share one 3-bank PSUM tile so exp runs as
        # one ACT instruction over [128, 1536]
        GROUPS = [3] * 10 + [2]
        if debug_taps:
            nc.sync.dma_start(out=dbg["qp"][:, :], in_=qp_sb)
            nc.sync.dma_start(out=dbg["kp"][:, :], in_=kp_sb)
            nc.sync.dma_start(out=dbg["v"][:, :], in_=v_sb)
        for q in range(NQC):
            qs = slice(q * QC, (q + 1) * QC)
            acc = psum.tile([VW, QC], F32, tag="acc", bufs=1)
            kbase = 0
            for gs in GROUPS:
                sc = psum.tile([128, gs * QC], F32, tag="s", bufs=2)
                for t in range(gs):
                    k = kbase + t
                    nc.tensor.matmul(
                        sc[:, t * QC:(t + 1) * QC],
                        kp_sb[:, k * KC:(k + 1) * KC],        # k'T chunk [64,128]
                        qp_sb[:, qs],                         # q'T [64,512]
                        start=True, stop=True)
                ex = work.tile([128, gs * QC], F32, tag="e", bufs=3)
                nc.scalar.activation(ex, sc, EXP)
                if debug_taps and q == 0 and kbase == 0:
                    nc.sync.dma_start(out=dbg["ex0"][:, :], in_=ex)
                for t in range(gs):
                    k = kbase + t
                    nc.tensor.matmul(
                        acc,
                        v_sb[:, k * VW:(k + 1) * VW],         # [128, 33]
                        ex[:, t * QC:(t + 1) * QC],           # [128, 512]
                        start=(k == 0), stop=(k == NKC - 1))
                kbase += gs

            # softmax normalization: rows 0-31 /= row 32
            if debug_taps and q == 0:
                acc_cp = work.tile([VW, QC], F32, tag="acccp", bufs=1)
                nc.vector.tensor_copy(acc_cp, acc)
                nc.sync.dma_start(out=dbg["acc0"][:, :], in_=acc_cp)
            recip = work.tile([1, QC], F32, tag="r", bufs=2)
            nc.vector.reciprocal(recip, acc[HD:HD + 1, :])
            rb = work.tile([HD, QC], F32, tag="rb", bufs=2)
            nc.gpsimd.partition_broadcast(rb, recip)
            outT = work.tile([HD, QC], F32, tag="o", bufs=2)
            nc.vector.tensor_mul(outT, acc[0:HD, :], rb)
            if debug_taps and q == 0:
                nc.sync.dma_start(out=dbg["recip0"][:, :], in_=recip)
                nc.sync.dma_start(out=dbg["rb0"][:, :], in_=rb)
                nc.sync.dma_start(out=dbg["outT0"][:, :], in_=outT)

            # final projection for this query chunk: 4 token-tiles of 128
            os4 = work.tile([128, 4, KEY_DIM], F32, tag="os", bufs=2)
            for t in range(4):
                pf = psum.tile([128, KEY_DIM], F32, tag="x", bufs=1)
                nc.tensor.matmul(pf, outT[:, t * 128:(t + 1) * 128], wf_sb,
                                 start=True, stop=True)
                nc.vector.tensor_copy(os4[:, t, :], pf)
            # one 512KB DMA out per query chunk
            out_view = out_d[:, :].rearrange("(q t p) c -> q p t c", t=4, p=128)
            nc.sync.dma_start(out=out_view[q], in_=os4)

    nc.compile()
    return nc


def _prepare_inputs(x, Wp, bp, Wf, bf):
    """Build per-core input maps (head h -> core h)."""
    x = np.ascontiguousarray(x, dtype=np.float32)
    Wp = np.ascontiguousarray(Wp, dtype=np.float32)
    bp = np.ascontiguousarray(bp, dtype=np.float32)
    Wf = np.ascontiguousarray(Wf, dtype=np.float32)
    bf = np.ascontiguousarray(bf, dtype=np.float32)

    r_w, theta = _polar_constants()
    inv_sqrt_hd = np.float32(1.0 / np.sqrt(np.float32(HD)))
    cos_t = np.cos(theta).astype(np.float32)
    sin_t = np.sin(theta).astype(np.float32)

    mcq = np.empty((64, N), dtype=np.float32)
    mcq[0:32, :] = cos_t * inv_sqrt_hd
    mcq[32:64, :] = sin_t * inv_sqrt_hd
    mck = np.empty((64, N), dtype=np.float32)
    mck[0:32, :] = r_w * cos_t
    mck[32:64, :] = r_w * sin_t

    xT = np.ascontiguousarray(x.reshape(N, C).T)  # [C, N]

    # NOTE: q/k biases (bp[0:512]) are NOT applied on device; they are zero
    # by the problem spec (fill=zeros). The v bias folds exactly into a
    # host-side output bias since softmax rows sum to 1:
    #   p @ (v + bv) @ Wf_h = p @ v @ Wf_h + bv @ Wf_h
    assert np.max(np.abs(bp[:2 * KEY_DIM])) == 0.0, "nonzero q/k bias unsupported"
    bv_full = bp[2 * KEY_DIM:3 * KEY_DIM]
    host_bias = (bf + bv_full @ Wf).astype(np.float32)  # [256]

    in_maps = []
    for h in range(NCORES):
        qs = slice(32 * h, 32 * h + 32)
        Wq = Wp[:, 0 * KEY_DIM:1 * KEY_DIM][:, qs]
        Wk = Wp[:, 1 * KEY_DIM:2 * KEY_DIM][:, qs]
        Wv = Wp[:, 2 * KEY_DIM:3 * KEY_DIM][:, qs]
        wqq = np.ascontiguousarray(np.concatenate([Wq, Wq], axis=1))  # [128, 64]
        wkk = np.ascontiguousarray(np.concatenate([Wk, Wk], axis=1))  # [128, 64]
        wf_h = np.ascontiguousarray(Wf[qs, :])                 # [32, 256]
        in_maps.append({
            "xT": xT, "mcq": mcq, "mck": mck,
            "wqq": wqq, "wkk": wkk,
            "wv": np.ascontiguousarray(Wv),
            "wf": wf_h,
        })
    return in_maps, host_bias


def kernel(x, Wp, bp, Wf, bf):
    from concourse.bass_utils import run_bass_kernel_spmd

    if "nc" not in _CACHE:
        _CACHE["nc"] = _build_nc()
    nc = _CACHE["nc"]

    in_maps, host_bias = _prepare_inputs(x, Wp, bp, Wf, bf)
    res = run_bass_kernel_spmd(nc, in_maps, core_ids=list(range(NCORES)))
    parts = [r["out"] for r in res.results]
    out = np.sum(np.stack(parts, axis=0), axis=0, dtype=np.float32)
    out = out + host_bias[None, :]
    return out.reshape(B, HI, WI, KEY_DIM).astype(np.float32)


# revision 21
# speedup vs baseline: 1.8474x; 1.8474x over previous
"""EnhancedPolarAttention Trainium2 Bass kernel.

Full inputs in, full output out. Head-parallel across 8 NeuronCores
(1 head per core). See bottom of file for the host-side kernel() entry.

Math: scores = (q.k)/sqrt(hd) * r_w[j] * cos(theta_i - theta_j)
   with cos(a-b) = cos a cos b + sin a sin b, this is exactly
   q'_i . k'_j  with
   q' = [q * cos(theta_i), q * sin(theta_i)] / sqrt(hd)   (64-dim)
   k' = [k * r_w cos(theta_j), k * r_w sin(theta_j)]      (64-dim)
so the polar modulation folds into the QK^T matmul (contraction 64).

Scores are tiny (|s| < ~0.5), so softmax needs no max subtraction:
exp on ScalarE directly, and the denominator Z = sum_j exp(s_ij) is
obtained for free by augmenting v with a ones column inside the
attn @ v matmul (out row 32 = Z).

Everything is computed in a transposed (feature-major) layout so no
on-device transposes are needed at all:
  xT [C=128, N=4096]          (host pre-transposed, replicated)
  qk' [128, 4096]             rows 0-63 = q'T, rows 64-127 = k'T
  scoresT tile [128 keys, 512 queries] = k'chunk.T @ q'T  (PSUM)
  acc [33, 512] += v_aug_chunk.T @ exp(scoresT chunk)     (PSUM)
  outT [32, 512] = acc[0:32] * broadcast(1/acc[32])
  final [128 tok, 256] = outT_slice.T @ Wf_h  (+ bf/8)
Host sums the 8 per-head partial projections.
"""

import numpy as np

# ---- problem constants (hardcoded per contract) ----
B, HI, WI, C = 1, 64, 64, 128
N = HI * WI            # 4096
KEY_DIM = 256
NH = 8                 # heads
HD = KEY_DIM // NH     # 32
NCORES = 8
QC = 512               # query chunk = one PSUM bank of f32
NQC = N // QC          # 8
KC = 128               # key chunk = partition dim
NKC = N // KC          # 32
VW = HD + 1            # v augmented with ones column -> 33

_CACHE = {}


def _polar_constants():
    """Match reference._polar_constants in float32 numpy."""
    H, W = HI, WI
    y, x = np.meshgrid(np.arange(H, dtype=np.float32),
                       np.arange(W, dtype=np.float32))
    x = x.reshape(-1)
    y = y.reshape(-1)
    r = np.sqrt(np.square(x - W / 2) + np.square(y - H / 2)).astype(np.float32) + np.float32(1e-6)
    theta = np.arctan2(y - H / 2, x - W / 2).astype(np.float32)
    log_r = (np.log(r) / np.log(r.max())).astype(np.float32)
    theta = ((theta + 2 * np.pi) % (2 * np.pi)).astype(np.float32)
    r_weight = (1.0 / (log_r + 1.0)).astype(np.float32)
    return r_weight, theta


def _build_nc(debug_taps=False):
    import concourse.mybir as mybir
    import concourse.tile as tile
    from concourse import bacc

    F32 = mybir.dt.float32
    BF16 = mybir.dt.bfloat16
    EXP = mybir.ActivationFunctionType.Exp

    nc = bacc.Bacc("TRN2", target_bir_lowering=False)
    dbg = {}
    if debug_taps:
        dbg["qp"] = nc.dram_tensor("dbg_qp", [64, N], F32, kind="ExternalOutput")
        dbg["kp"] = nc.dram_tensor("dbg_kp", [64, N], F32, kind="ExternalOutput")
        dbg["v"] = nc.dram_tensor("dbg_v", [128, NKC * VW], F32, kind="ExternalOutput")
        dbg["ex0"] = nc.dram_tensor("dbg_ex0", [128, 3 * QC], F32, kind="ExternalOutput")
        dbg["acc0"] = nc.dram_tensor("dbg_acc0", [VW, QC], F32, kind="ExternalOutput")
        dbg["recip0"] = nc.dram_tensor("dbg_recip0", [1, QC], F32, kind="ExternalOutput")
        dbg["rb0"] = nc.dram_tensor("dbg_rb0", [HD, QC], F32, kind="ExternalOutput")
        dbg["outT0"] = nc.dram_tensor("dbg_outT0", [HD, QC], F32, kind="ExternalOutput")

    xT_d = nc.dram_tensor("xT", [C, N], F32, kind="ExternalInput")
    mcq_d = nc.dram_tensor("mcq", [64, N], F32, kind="ExternalInput")
    mck_d = nc.dram_tensor("mck", [64, N], F32, kind="ExternalInput")
    wqq_d = nc.dram_tensor("wqq", [C, 64], F32, kind="ExternalInput")
    wkk_d = nc.dram_tensor("wkk", [C, 64], F32, kind="ExternalInput")
    wv_d = nc.dram_tensor("wv", [C, HD], F32, kind="ExternalInput")
    wf_d = nc.dram_tensor("wf", [HD, KEY_DIM], F32, kind="ExternalInput")
    out_d = nc.dram_tensor("out", [N, KEY_DIM], F32, kind="ExternalOutput")

    with tile.TileContext(nc) as tc, \
         tc.tile_pool(name="singles", bufs=1) as singles, \
         tc.tile_pool(name="work", bufs=2) as work, \
         tc.tile_pool(name="psum", bufs=2, space="PSUM") as psum:

        # ---- persistent SBUF ----
        xT_sb = singles.tile([C, N], F32)
        mcq_sb = singles.tile([64, N], F32)
        mck_sb = singles.tile([64, N], F32)
        qp_sb = singles.tile([64, N], BF16)       # q'T
        kp_sb = singles.tile([64, N], BF16)       # k'T
        v_sb = singles.tile([128, NKC * VW], BF16)  # 32 chunks of [128, 33]
        wqq_sb = singles.tile([C, 64], F32)
        wkk_sb = singles.tile([C, 64], F32)
        wv_sb = singles.tile([C, HD], F32)
        wf_sb = singles.tile([HD, KEY_DIM], F32)

        nc.vector.memset(v_sb, 1.0)   # every 33rd column stays 1.0

        # ---- input DMAs (chunked so compute can start early) ----
        for i in range(NQC):
            s = slice(i * QC, (i + 1) * QC)
            nc.sync.dma_start(out=xT_sb[:, s], in_=xT_d[:, s])
            nc.sync.dma_start(out=mcq_sb[:, s], in_=mcq_d[:, s])
            nc.sync.dma_start(out=mck_sb[:, s], in_=mck_d[:, s])
        nc.sync.dma_start(out=wqq_sb, in_=wqq_d[:, :])
        nc.sync.dma_start(out=wkk_sb, in_=wkk_d[:, :])
        nc.sync.dma_start(out=wv_sb, in_=wv_d[:, :])
        nc.sync.dma_start(out=wf_sb, in_=wf_d[:, :])

        # ---- phase A: q'/k' projection with polar modulation fused ----
        # ps_q = Wqq.T @ xT -> [64 feat(q,q), 512 tok]; q' = (ps_q+bq)*mcq
        # mcq rows: cos/sqrt(hd) x32, sin/sqrt(hd) x32; mck: r*cos, r*sin
        for i in range(NQC):
            s = slice(i * QC, (i + 1) * QC)
            ps_q = psum.tile([64, QC], F32, tag="s")
            nc.tensor.matmul(ps_q, wqq_sb, xT_sb[:, s], start=True, stop=True)
            nc.vector.tensor_mul(qp_sb[:, s], ps_q, mcq_sb[:, s])
            ps_k = psum.tile([64, QC], F32, tag="s")
            nc.tensor.matmul(ps_k, wkk_sb, xT_sb[:, s], start=True, stop=True)
            nc.vector.tensor_mul(kp_sb[:, s], ps_k, mck_sb[:, s])

        # ---- phase A2: v projection (token-major directly) ----
        # v_chunk [128 tok, 32] = xT_chunk.T @ Wv ; +bias, into v_sb 33-blocks
        for j in range(NKC):
            ps_v = psum.tile([128, HD], F32, tag="s")
            nc.tensor.matmul(ps_v, xT_sb[:, j * KC:(j + 1) * KC], wv_sb,
                             start=True, stop=True)
            nc.vector.tensor_copy(v_sb[:, j * VW:j * VW + HD], ps_v)

        # ---- phase B: attention main loop ----
        # groups of 3 key-chunks share one 3-bank PSUM tile so exp runs as
        # one ACT instruction over [128, 1536]
        GROUPS = [3] * 10 + [2]
        if debug_taps:
            nc.gpsimd.dma_start(out=dbg["qp"][:, :], in_=qp_sb)
            nc.gpsimd.dma_start(out=dbg["kp"][:, :], in_=kp_sb)
            nc.gpsimd.dma_start(out=dbg["v"][:, :], in_=v_sb)
        for q in range(NQC):
            qs = slice(q * QC, (q + 1) * QC)
            acc = psum.tile([VW, QC], F32, tag="acc", bufs=1)
            kbase = 0
            for gs in GROUPS:
                sc = psum.tile([128, gs * QC], F32, tag="s", bufs=2)
                for t in range(gs):
                    k = kbase + t
                    nc.tensor.matmul(
                        sc[:, t * QC:(t + 1) * QC],
                        kp_sb[:, k * KC:(k + 1) * KC],        # k'T chunk [64,128]
                        qp_sb[:, qs],                         # q'T [64,512]
                        start=True, stop=True)
                ex = work.tile([128, gs * QC], BF16, tag="e", bufs=3)
                nc.scalar.activation(ex, sc, EXP)
                if debug_taps and q == 0 and kbase == 0:
                    nc.gpsimd.dma_start(out=dbg["ex0"][:, :], in_=ex)
                for t in range(gs):
                    k = kbase + t
                    nc.tensor.matmul(
                        acc,
                        v_sb[:, k * VW:(k + 1) * VW],         # [128, 33]
                        ex[:, t * QC:(t + 1) * QC],           # [128, 512]
                        start=(k == 0), stop=(k == NKC - 1))
                kbase += gs

            # softmax normalization: rows 0-31 /= row 32.
            # Copy acc out of PSUM first: frees the accumulator bank for the
            # next query chunk after ~0.6us instead of the ~5us recip chain.
            accs = work.tile([VW, QC], F32, tag="accs", bufs=2)
            nc.vector.tensor_copy(accs, acc)
            if debug_taps and q == 0:
                nc.sync.dma_start(out=dbg["acc0"][:, :], in_=accs)
            recip = work.tile([1, QC], F32, tag="r", bufs=2)
            nc.vector.reciprocal(recip, accs[HD:HD + 1, :])
            rb = work.tile([HD, QC], F32, tag="rb", bufs=2)
            nc.gpsimd.partition_broadcast(rb, recip)
            outT = work.tile([HD, QC], F32, tag="o", bufs=2)
            nc.vector.tensor_mul(outT, accs[0:HD, :], rb)
            if debug_taps and q == 0:
                nc.sync.dma_start(out=dbg["recip0"][:, :], in_=recip)
                nc.sync.dma_start(out=dbg["rb0"][:, :], in_=rb)
                nc.sync.dma_start(out=dbg["outT0"][:, :], in_=outT)

            # final projection for this query chunk: 4 token-tiles of 128
            os4 = work.tile([128, 4, KEY_DIM], F32, tag="os", bufs=2)
            for t in range(4):
                pf = psum.tile([128, KEY_DIM], F32, tag="x", bufs=1)
                nc.tensor.matmul(pf, outT[:, t * 128:(t + 1) * 128], wf_sb,
                                 start=True, stop=True)
                nc.vector.tensor_copy(os4[:, t, :], pf)
            # one 512KB DMA out per query chunk
            out_view = out_d[:, :].rearrange("(q t p) c -> q p t c", t=4, p=128)
            nc.sync.dma_start(out=out_view[q], in_=os4)

    nc.compile()
    return nc


def _prepare_inputs(x, Wp, bp, Wf, bf):
    """Build per-core input maps (head h -> core h)."""
    x = np.ascontiguousarray(x, dtype=np.float32)
    Wp = np.ascontiguousarray(Wp, dtype=np.float32)
    bp = np.ascontiguousarray(bp, dtype=np.float32)
    Wf = np.ascontiguousarray(Wf, dtype=np.float32)
    bf = np.ascontiguousarray(bf, dtype=np.float32)

    r_w, theta = _polar_constants()
    inv_sqrt_hd = np.float32(1.0 / np.sqrt(np.float32(HD)))
    cos_t = np.cos(theta).astype(np.float32)
    sin_t = np.sin(theta).astype(np.float32)

    mcq = np.empty((64, N), dtype=np.float32)
    mcq[0:32, :] = cos_t * inv_sqrt_hd
    mcq[32:64, :] = sin_t * inv_sqrt_hd
    mck = np.empty((64, N), dtype=np.float32)
    mck[0:32, :] = r_w * cos_t
    mck[32:64, :] = r_w * sin_t

    xT = np.ascontiguousarray(x.reshape(N, C).T)  # [C, N]

    # NOTE: q/k biases (bp[0:512]) are NOT applied on device; they are zero
    # by the problem spec (fill=zeros). The v bias folds exactly into a
    # host-side output bias since softmax rows sum to 1:
    #   p @ (v + bv) @ Wf_h = p @ v @ Wf_h + bv @ Wf_h
    assert np.max(np.abs(bp[:2 * KEY_DIM])) == 0.0, "nonzero q/k bias unsupported"
    bv_full = bp[2 * KEY_DIM:3 * KEY_DIM]
    host_bias = (bf + bv_full @ Wf).astype(np.float32)  # [256]

    in_maps = []
    for h in range(NCORES):
        qs = slice(32 * h, 32 * h + 32)
        Wq = Wp[:, 0 * KEY_DIM:1 * KEY_DIM][:, qs]
        Wk = Wp[:, 1 * KEY_DIM:2 * KEY_DIM][:, qs]
        Wv = Wp[:, 2 * KEY_DIM:3 * KEY_DIM][:, qs]
        wqq = np.ascontiguousarray(np.concatenate([Wq, Wq], axis=1))  # [128, 64]
        wkk = np.ascontiguousarray(np.concatenate([Wk, Wk], axis=1))  # [128, 64]
        wf_h = np.ascontiguousarray(Wf[qs, :])                 # [32, 256]
        in_maps.append({
            "xT": xT, "mcq": mcq, "mck": mck,
            "wqq": wqq, "wkk": wkk,
            "wv": np.ascontiguousarray(Wv),
            "wf": wf_h,
        })
    return in_maps, host_bias


def kernel(x, Wp, bp, Wf, bf):
    from concourse.bass_utils import run_bass_kernel_spmd

    if "nc" not in _CACHE:
        _CACHE["nc"] = _build_nc()
    nc = _CACHE["nc"]

    in_maps, host_bias = _prepare_inputs(x, Wp, bp, Wf, bf)
    res = run_bass_kernel_spmd(nc, in_maps, core_ids=list(range(NCORES)))
    parts = [r["out"] for r in res.results]
    out = np.sum(np.stack(parts, axis=0), axis=0, dtype=np.float32)
    out = out + host_bias[None, :]
    return out.reshape(B, HI, WI, KEY_DIM).astype(np.float32)


# revision 22
# speedup vs baseline: 1.8747x; 1.0148x over previous
"""EnhancedPolarAttention Trainium2 Bass kernel.

Full inputs in, full output out. Head-parallel across 8 NeuronCores
(1 head per core). See bottom of file for the host-side kernel() entry.

Math: scores = (q.k)/sqrt(hd) * r_w[j] * cos(theta_i - theta_j)
   with cos(a-b) = cos a cos b + sin a sin b, this is exactly
   q'_i . k'_j  with
   q' = [q * cos(theta_i), q * sin(theta_i)] / sqrt(hd)   (64-dim)
   k' = [k * r_w cos(theta_j), k * r_w sin(theta_j)]      (64-dim)
so the polar modulation folds into the QK^T matmul (contraction 64).

Scores are tiny (|s| < ~0.5), so softmax needs no max subtraction:
exp on ScalarE directly, and the denominator Z = sum_j exp(s_ij) is
obtained for free by augmenting v with a ones column inside the
attn @ v matmul (out row 32 = Z).

Everything is computed in a transposed (feature-major) layout so no
on-device transposes are needed at all:
  xT [C=128, N=4096]          (host pre-transposed, replicated)
  qk' [128, 4096]             rows 0-63 = q'T, rows 64-127 = k'T
  scoresT tile [128 keys, 512 queries] = k'chunk.T @ q'T  (PSUM)
  acc [33, 512] += v_aug_chunk.T @ exp(scoresT chunk)     (PSUM)
  outT [32, 512] = acc[0:32] * broadcast(1/acc[32])
  final [128 tok, 256] = outT_slice.T @ Wf_h  (+ bf/8)
Host sums the 8 per-head partial projections.
"""

import numpy as np

# ---- problem constants (hardcoded per contract) ----
B, HI, WI, C = 1, 64, 64, 128
N = HI * WI            # 4096
KEY_DIM = 256
NH = 8                 # heads
HD = KEY_DIM // NH     # 32
NCORES = 8
QC = 512               # query chunk = one PSUM bank of f32
NQC = N // QC          # 8
KC = 128               # key chunk = partition dim
NKC = N // KC          # 32
VW = HD + 1            # v augmented with ones column -> 33

_CACHE = {}


def _polar_constants():
    """Match reference._polar_constants in float32 numpy."""
    H, W = HI, WI
    y, x = np.meshgrid(np.arange(H, dtype=np.float32),
                       np.arange(W, dtype=np.float32))
    x = x.reshape(-1)
    y = y.reshape(-1)
    r = np.sqrt(np.square(x - W / 2) + np.square(y - H / 2)).astype(np.float32) + np.float32(1e-6)
    theta = np.arctan2(y - H / 2, x - W / 2).astype(np.float32)
    log_r = (np.log(r) / np.log(r.max())).astype(np.float32)
    theta = ((theta + 2 * np.pi) % (2 * np.pi)).astype(np.float32)
    r_weight = (1.0 / (log_r + 1.0)).astype(np.float32)
    return r_weight, theta


def _build_nc(debug_taps=False):
    import concourse.mybir as mybir
    import concourse.tile as tile
    from concourse import bacc

    F32 = mybir.dt.float32
    BF16 = mybir.dt.float16  # fp16: same PE speed as bf16, 8x the mantissa
    EXP = mybir.ActivationFunctionType.Exp

    nc = bacc.Bacc("TRN2", target_bir_lowering=False)
    dbg = {}
    if debug_taps:
        dbg["qp"] = nc.dram_tensor("dbg_qp", [64, N], F32, kind="ExternalOutput")
        dbg["kp"] = nc.dram_tensor("dbg_kp", [64, N], F32, kind="ExternalOutput")
        dbg["v"] = nc.dram_tensor("dbg_v", [128, NKC * VW], F32, kind="ExternalOutput")
        dbg["ex0"] = nc.dram_tensor("dbg_ex0", [128, 3 * QC], F32, kind="ExternalOutput")
        dbg["acc0"] = nc.dram_tensor("dbg_acc0", [VW, QC], F32, kind="ExternalOutput")
        dbg["recip0"] = nc.dram_tensor("dbg_recip0", [1, QC], F32, kind="ExternalOutput")
        dbg["rb0"] = nc.dram_tensor("dbg_rb0", [HD, QC], F32, kind="ExternalOutput")
        dbg["outT0"] = nc.dram_tensor("dbg_outT0", [HD, QC], F32, kind="ExternalOutput")

    xT_d = nc.dram_tensor("xT", [C, N], F32, kind="ExternalInput")
    mcq_d = nc.dram_tensor("mcq", [64, N], F32, kind="ExternalInput")
    mck_d = nc.dram_tensor("mck", [64, N], F32, kind="ExternalInput")
    wqq_d = nc.dram_tensor("wqq", [C, 64], F32, kind="ExternalInput")
    wkk_d = nc.dram_tensor("wkk", [C, 64], F32, kind="ExternalInput")
    wv_d = nc.dram_tensor("wv", [C, HD], F32, kind="ExternalInput")
    wf_d = nc.dram_tensor("wf", [HD, KEY_DIM], F32, kind="ExternalInput")
    out_d = nc.dram_tensor("out", [N, KEY_DIM], F32, kind="ExternalOutput")

    with tile.TileContext(nc) as tc, \
         tc.tile_pool(name="singles", bufs=1) as singles, \
         tc.tile_pool(name="work", bufs=2) as work, \
         tc.tile_pool(name="psum", bufs=2, space="PSUM") as psum:

        # ---- persistent SBUF ----
        xT_sb = singles.tile([C, N], F32)
        mcq_sb = singles.tile([64, N], F32)
        mck_sb = singles.tile([64, N], F32)
        qp_sb = singles.tile([64, N], BF16)       # q'T
        kp_sb = singles.tile([64, N], BF16)       # k'T
        v_sb = singles.tile([128, NKC * VW], BF16)  # 32 chunks of [128, 33]
        wqq_sb = singles.tile([C, 64], F32)
        wkk_sb = singles.tile([C, 64], F32)
        wv_sb = singles.tile([C, HD], F32)
        wf_sb = singles.tile([HD, KEY_DIM], F32)

        nc.vector.memset(v_sb, 1.0)   # every 33rd column stays 1.0

        # ---- input DMAs (chunked so compute can start early) ----
        for i in range(NQC):
            s = slice(i * QC, (i + 1) * QC)
            nc.sync.dma_start(out=xT_sb[:, s], in_=xT_d[:, s])
            nc.sync.dma_start(out=mcq_sb[:, s], in_=mcq_d[:, s])
            nc.sync.dma_start(out=mck_sb[:, s], in_=mck_d[:, s])
        nc.sync.dma_start(out=wqq_sb, in_=wqq_d[:, :])
        nc.sync.dma_start(out=wkk_sb, in_=wkk_d[:, :])
        nc.sync.dma_start(out=wv_sb, in_=wv_d[:, :])
        nc.sync.dma_start(out=wf_sb, in_=wf_d[:, :])

        # ---- phase A: q'/k' projection with polar modulation fused ----
        # ps_q = Wqq.T @ xT -> [64 feat(q,q), 512 tok]; q' = (ps_q+bq)*mcq
        # mcq rows: cos/sqrt(hd) x32, sin/sqrt(hd) x32; mck: r*cos, r*sin
        for i in range(NQC):
            s = slice(i * QC, (i + 1) * QC)
            ps_q = psum.tile([64, QC], F32, tag="s")
            nc.tensor.matmul(ps_q, wqq_sb, xT_sb[:, s], start=True, stop=True)
            nc.vector.tensor_mul(qp_sb[:, s], ps_q, mcq_sb[:, s])
            ps_k = psum.tile([64, QC], F32, tag="s")
            nc.tensor.matmul(ps_k, wkk_sb, xT_sb[:, s], start=True, stop=True)
            nc.vector.tensor_mul(kp_sb[:, s], ps_k, mck_sb[:, s])

        # ---- phase A2: v projection (token-major directly) ----
        # v_chunk [128 tok, 32] = xT_chunk.T @ Wv ; +bias, into v_sb 33-blocks
        for j in range(NKC):
            ps_v = psum.tile([128, HD], F32, tag="s")
            nc.tensor.matmul(ps_v, xT_sb[:, j * KC:(j + 1) * KC], wv_sb,
                             start=True, stop=True)
            nc.vector.tensor_copy(v_sb[:, j * VW:j * VW + HD], ps_v)

        # ---- phase B: attention main loop ----
        # groups of 3 key-chunks share one 3-bank PSUM tile so exp runs as
        # one ACT instruction over [128, 1536]
        GROUPS = [3] * 10 + [2]
        if debug_taps:
            nc.gpsimd.dma_start(out=dbg["qp"][:, :], in_=qp_sb)
            nc.gpsimd.dma_start(out=dbg["kp"][:, :], in_=kp_sb)
            nc.gpsimd.dma_start(out=dbg["v"][:, :], in_=v_sb)
        for q in range(NQC):
            qs = slice(q * QC, (q + 1) * QC)
            acc = psum.tile([VW, QC], F32, tag="acc", bufs=1)
            kbase = 0
            for gs in GROUPS:
                sc = psum.tile([128, gs * QC], F32, tag="s", bufs=2)
                for t in range(gs):
                    k = kbase + t
                    nc.tensor.matmul(
                        sc[:, t * QC:(t + 1) * QC],
                        kp_sb[:, k * KC:(k + 1) * KC],        # k'T chunk [64,128]
                        qp_sb[:, qs],                         # q'T [64,512]
                        start=True, stop=True)
                ex = work.tile([128, gs * QC], BF16, tag="e", bufs=3)
                nc.scalar.activation(ex, sc, EXP)
                if debug_taps and q == 0 and kbase == 0:
                    nc.gpsimd.dma_start(out=dbg["ex0"][:, :], in_=ex)
                for t in range(gs):
                    k = kbase + t
                    nc.tensor.matmul(
                        acc,
                        v_sb[:, k * VW:(k + 1) * VW],         # [128, 33]
                        ex[:, t * QC:(t + 1) * QC],           # [128, 512]
                        start=(k == 0), stop=(k == NKC - 1))
                kbase += gs

            # softmax normalization: rows 0-31 /= row 32.
            # Copy acc out of PSUM first: frees the accumulator bank for the
            # next query chunk after ~0.6us instead of the ~5us recip chain.
            accs = work.tile([VW, QC], F32, tag="accs", bufs=2)
            nc.vector.tensor_copy(accs, acc)
            if debug_taps and q == 0:
                nc.sync.dma_start(out=dbg["acc0"][:, :], in_=accs)
            recip = work.tile([1, QC], F32, tag="r", bufs=2)
            nc.vector.reciprocal(recip, accs[HD:HD + 1, :])
            rb = work.tile([HD, QC], F32, tag="rb", bufs=2)
            nc.gpsimd.partition_broadcast(rb, recip)
            outT = work.tile([HD, QC], F32, tag="o", bufs=2)
            nc.vector.tensor_mul(outT, accs[0:HD, :], rb)
            if debug_taps and q == 0:
                nc.sync.dma_start(out=dbg["recip0"][:, :], in_=recip)
                nc.sync.dma_start(out=dbg["rb0"][:, :], in_=rb)
                nc.sync.dma_start(out=dbg["outT0"][:, :], in_=outT)

            # final projection for this query chunk: 4 token-tiles of 128
            os4 = work.tile([128, 4, KEY_DIM], F32, tag="os", bufs=2)
            for t in range(4):
                pf = psum.tile([128, KEY_DIM], F32, tag="x", bufs=1)
                nc.tensor.matmul(pf, outT[:, t * 128:(t + 1) * 128], wf_sb,
                                 start=True, stop=True)
                nc.vector.tensor_copy(os4[:, t, :], pf)
            # one 512KB DMA out per query chunk
            out_view = out_d[:, :].rearrange("(q t p) c -> q p t c", t=4, p=128)
            nc.sync.dma_start(out=out_view[q], in_=os4)

    nc.compile()
    return nc


def _prepare_inputs(x, Wp, bp, Wf, bf):
    """Build per-core input maps (head h -> core h)."""
    x = np.ascontiguousarray(x, dtype=np.float32)
    Wp = np.ascontiguousarray(Wp, dtype=np.float32)
    bp = np.ascontiguousarray(bp, dtype=np.float32)
    Wf = np.ascontiguousarray(Wf, dtype=np.float32)
    bf = np.ascontiguousarray(bf, dtype=np.float32)

    r_w, theta = _polar_constants()
    inv_sqrt_hd = np.float32(1.0 / np.sqrt(np.float32(HD)))
    cos_t = np.cos(theta).astype(np.float32)
    sin_t = np.sin(theta).astype(np.float32)

    mcq = np.empty((64, N), dtype=np.float32)
    mcq[0:32, :] = cos_t * inv_sqrt_hd
    mcq[32:64, :] = sin_t * inv_sqrt_hd
    mck = np.empty((64, N), dtype=np.float32)
    mck[0:32, :] = r_w * cos_t
    mck[32:64, :] = r_w * sin_t

    xT = np.ascontiguousarray(x.reshape(N, C).T)  # [C, N]

    # NOTE: q/k biases (bp[0:512]) are NOT applied on device; they are zero
    # by the problem spec (fill=zeros). The v bias folds exactly into a
    # host-side output bias since softmax rows sum to 1:
    #   p @ (v + bv) @ Wf_h = p @ v @ Wf_h + bv @ Wf_h
    assert np.max(np.abs(bp[:2 * KEY_DIM])) == 0.0, "nonzero q/k bias unsupported"
    bv_full = bp[2 * KEY_DIM:3 * KEY_DIM]
    host_bias = (bf + bv_full @ Wf).astype(np.float32)  # [256]

    in_maps = []
    for h in range(NCORES):
        qs = slice(32 * h, 32 * h + 32)
        Wq = Wp[:, 0 * KEY_DIM:1 * KEY_DIM][:, qs]
        Wk = Wp[:, 1 * KEY_DIM:2 * KEY_DIM][:, qs]
        Wv = Wp[:, 2 * KEY_DIM:3 * KEY_DIM][:, qs]
        wqq = np.ascontiguousarray(np.concatenate([Wq, Wq], axis=1))  # [128, 64]
        wkk = np.ascontiguousarray(np.concatenate([Wk, Wk], axis=1))  # [128, 64]
        wf_h = np.ascontiguousarray(Wf[qs, :])                 # [32, 256]
        in_maps.append({
            "xT": xT, "mcq": mcq, "mck": mck,
            "wqq": wqq, "wkk": wkk,
            "wv": np.ascontiguousarray(Wv),
            "wf": wf_h,
        })
    return in_maps, host_bias


def kernel(x, Wp, bp, Wf, bf):
    from concourse.bass_utils import run_bass_kernel_spmd

    if "nc" not in _CACHE:
        _CACHE["nc"] = _build_nc()
    nc = _CACHE["nc"]

    in_maps, host_bias = _prepare_inputs(x, Wp, bp, Wf, bf)
    res = run_bass_kernel_spmd(nc, in_maps, core_ids=list(range(NCORES)))
    parts = [r["out"] for r in res.results]
    out = np.sum(np.stack(parts, axis=0), axis=0, dtype=np.float32)
    out = out + host_bias[None, :]
    return out.reshape(B, HI, WI, KEY_DIM).astype(np.float32)
